# revision 26
# baseline (speedup 1.0000x reference)
"""Trainium2 Bass kernel for nn_CleanupBlock — token-partition redesign.

Math (folded, identical to baseline): per layer l the softmax numerators e_l
are carried unnormalized; consecutive layers fold into [256,512] operators
  g_l = e_{l-1} @ [M_l | G_{l-1}],  M_l = V_{l-1} K_l^T,  G = V V^T,
  scores_l = g_l[:, :256] / sqrt(e G e^T),  out = (e_4 / sum e_4) @ V_4.

Layout: TOKENS ON PARTITIONS for all activations ([128 tok, support/d] tiles).
This makes every per-token scalar (1/||x||, 1/sqrt(eGe), 1/sum e) a
per-partition scalar, so:
  - the score normalization fuses into the ACT exp via its per-partition
    `scale=` operand (no DVE multiply, no broadcast),
  - softmax denominators come free from exp's `accum_out`,
  - eGe comes from one fused DVE scalar_tensor_tensor (accum_out) per group,
  - ||x||^2 comes from 1-row matmuls (stationary x^2, moving ones),
  - 1/sum(e4) folds into the PSUM->SBUF output copies as a per-partition
    scale, so the final normalization costs nothing extra.
The price is a PE transpose + PSUM->SBUF copy of each e (contraction dim must
sit on partitions for the next matmul); that costs ~12% of PE and is far
cheaper than the broadcast/multiply traffic it removes.

Engines: PE matmuls/transposes; ACT exps + ln/exp rsqrt chain; DVE eT copies,
fused eGe reduce, reciprocal, half the out copies; GPSIMD x^2; SP(sync)
issues all DMAs (idle HWDGE queue). All matmul operands bf16 (full rate,
like f32r per the cost model, but halves DMA and enables DVE 2x modes).

Emission is modulo-scheduled: 8 token-pair streams staggered one pipeline
stage apart (stages S, L2, L3, L4, O1, O2 merged into 5 slot-groups), so
every in-order engine queue always has another stream's ready work queued
behind any semaphore wait. PSUM: 6-bank ring for scores/g/out tiles + 2-bank
ring for transpose staging.

Sharding: data-parallel over tokens, 2048 tokens/core, operators replicated.
"""

import numpy as np
import ml_dtypes

import concourse.bacc as bacc
import concourse.tile as tile
from concourse import mybir
from concourse.bass_utils import run_bass_kernel_spmd

F32 = mybir.dt.float32
BF16 = mybir.dt.bfloat16
NPBF16 = ml_dtypes.bfloat16

B, S, D = 4, 4096, 1024
NS = 256
NCORES = 8
T = (B * S) // NCORES   # 2048 tokens/core
TC = 512                # token chunk
NCH = T // TC           # 4 chunks
DC = D // 128           # 8 contraction chunks over D
NM = NS // 128          # 2 contraction chunks over supports

_CACHE = {}
import os
_PSS = int(os.environ.get('PSS', 99))


def _os2env(k, d):
    return os.environ.get(k, d)
_PSG = int(os.environ.get('PSG', 6))
_PSO = int(os.environ.get('PSO', 2))


def _patch_act_tables():
    """Pin Exp/Ln/Square to the combined natural_log_exp_and_others ACT table
    so the whole kernel needs a single ACT_TABLE_LOAD."""
    import concourse.bacc as _bacc
    if getattr(_bacc, "_act_tables_patched", False):
        return
    _orig = _bacc.get_activation_tables
    _special = {
        mybir.ActivationFunctionType.Exp,
        mybir.ActivationFunctionType.Ln,
        mybir.ActivationFunctionType.Square,
    }

    def _patched(module_arch):
        tabs = _orig(module_arch)
        return {
            name: (funcs if name == "natural_log_exp_and_others"
                   else funcs - _special)
            for name, funcs in tabs.items()
        }

    _bacc.get_activation_tables = _patched
    _bacc._act_tables_patched = True


def _build():
    _patch_act_tables()
    nc = bacc.Bacc("TRN2", target_bir_lowering=False, debug=False,
                   num_devices=NCORES)

    xt_d = nc.dram_tensor("xt", [D, T], BF16, kind="ExternalInput")
    k1t_d = nc.dram_tensor("k1t", [D, NS], BF16, kind="ExternalInput")
    mg_d = nc.dram_tensor("mg", [3, NS, 512], BF16, kind="ExternalInput")
    v4_d = nc.dram_tensor("v4", [NS, D], BF16, kind="ExternalInput")
    idm_d = nc.dram_tensor("idm", [128, 128], BF16, kind="ExternalInput")
    out_d = nc.dram_tensor("outt", [T, D], BF16, kind="ExternalOutput")

    Exp = mybir.ActivationFunctionType.Exp
    Ln = mybir.ActivationFunctionType.Ln
    MUL = mybir.AluOpType.mult
    ADD = mybir.AluOpType.add

    with tile.TileContext(nc) as tc:
        with (
            tc.tile_pool(name="wp", bufs=1) as wp,
            tc.tile_pool(name="xp", bufs=4) as xp,
            tc.tile_pool(name="x2p", bufs=4) as x2p,
            tc.tile_pool(name="ep", bufs=20) as ep,
            tc.tile_pool(name="etp", bufs=8) as etp,
            tc.tile_pool(name="scrp", bufs=8) as scrp,
            tc.tile_pool(name="rsp", bufs=28) as rsp,
            tc.tile_pool(name="op", bufs=6) as op,
            tc.tile_pool(name="psG", bufs=_PSG, space="PSUM") as psG,
            tc.tile_pool(name="psO", bufs=_PSO, space="PSUM") as psO,
        ):
            if _PSS == 0:
                psS, s_tag = psO, "o"
            elif _PSS == 99:
                psS, s_tag = psG, "g"
            else:
                psS, s_tag = psO, "o"  # placeholder
            # ---- first x chunk + k1 in interleaved halves: the first s1
            # matmuls only need the dc<4 halves of both, so PE starts ~2.5us
            # earlier than with monolithic loads ----
            xc0 = xp.tile([128, DC, TC], BF16, tag="x")
            k1 = wp.tile([128, DC, NS], BF16, tag="k1")
            xt_r = xt_d.ap()[:, 0:TC].rearrange("(dc p) t -> p dc t", p=128)
            k1_r = k1t_d.ap().rearrange("(c p) n -> p c n", p=128)
            nc.sync.dma_start(out=xc0[:, :, 0:256], in_=xt_r[:, :, 0:256])
            nc.sync.dma_start(out=k1, in_=k1_r)
            nc.sync.dma_start(out=xc0[:, :, 256:512], in_=xt_r[:, :, 256:512])
            mg = wp.tile([128, 3, NM, 512], BF16, tag="mg")
            nc.sync.dma_start(
                out=mg, in_=mg_d.ap().rearrange("l (m p) j -> p l m j", p=128))
            v4 = wp.tile([128, NM, D], BF16, tag="v4")
            nc.sync.dma_start(
                out=v4, in_=v4_d.ap().rearrange("(m p) d -> p m d", p=128))
            idm = wp.tile([128, 128], BF16, tag="idm")
            nc.sync.dma_start(out=idm, in_=idm_d.ap())
            ones = wp.tile([128, 1], BF16, tag="ones")
            nc.vector.memset(ones, 1.0)
            _nwarm = int(_os2env("NWARM", "0"))
            if _nwarm:
                # p-state warmup: PE ramps to full clock only after 3us of
                # continuous busy (first-busy-instant anchored). Dummy
                # matmuls during the initial DMA wait move pe_busy_start to
                # ~0.7us so all real matmuls run at full speed.
                wz = wp.tile([128, 512], BF16, tag="wz")
                nc.vector.memset(wz, 0.0)
                wps = psO.tile([128, 512], F32, tag="o", name="warm")
                for i in range(_nwarm):
                    nc.tensor.matmul(wps, wz[:, 0:128], wz,
                                     start=(i == 0), stop=(i == _nwarm - 1))

            xcs, x2s, rs1s = {}, {}, {}

            def chunk_load(c):
                tsl = slice(c * TC, (c + 1) * TC)
                if c == 0:
                    xc = xc0
                else:
                    xc = xp.tile([128, DC, TC], BF16, tag="x")
                    nc.sync.dma_start(
                        out=xc,
                        in_=xt_d.ap()[:, tsl].rearrange("(dc p) t -> p dc t",
                                                        p=128))
                x2 = x2p.tile([128, DC, TC], BF16, tag="x2")
                for g in range(4):
                    gsl = slice(g * 128, (g + 1) * 128)
                    if (c == 0 or (c == 1 and g < int(_os2env("X2DVE", "3")))
                            or (c >= 2 and g < int(_os2env("X2DVE2", "0")))):
                        # latency-critical early pieces on DVE (pipeline is
                        # filling); the rest on GPSIMD, keeping DVE free for
                        # steady-state chain work
                        nc.vector.tensor_mul(x2[:, :, gsl], xc[:, :, gsl],
                                             xc[:, :, gsl])
                    elif c >= 2 and g < int(_os2env("X2ACT", "0")):
                        # ACT Square is in the pinned exp/ln table; ACT has
                        # headroom in the 10-35us window where gpsimd pegs
                        nc.scalar.activation(
                            x2[:, :, gsl], xc[:, :, gsl],
                            mybir.ActivationFunctionType.Square)
                    else:
                        nc.gpsimd.tensor_mul(x2[:, :, gsl], xc[:, :, gsl],
                                             xc[:, :, gsl])
                xcs[c], x2s[c] = xc, x2

            def chunk_norm(c, half=None):
                # half=None: all 4 groups, one [128,4] rs1 tile.
                # half=0/1: one pair's groups (chunk-0 startup latency path).
                x2 = x2s[c]
                gl = (list(range(4)) if half is None
                      else [2 * half, 2 * half + 1])
                w = len(gl)
                n2p = psS.tile([128, w], F32, tag=s_tag, name="n2")
                for i, g in enumerate(gl):
                    gsl = slice(g * 128, (g + 1) * 128)
                    for dc in range(DC):
                        nc.tensor.matmul(
                            n2p[:, i:i + 1], x2[:, dc, gsl], ones,
                            start=(dc == 0), stop=(dc == DC - 1))
                ln1 = rsp.tile([128, w], F32, tag="rs", name="ln1")
                nc.scalar.activation(ln1, n2p, Ln)
                rs1 = rsp.tile([128, w], F32, tag="rs", name="rs1")
                nc.scalar.activation(rs1, ln1, Exp, scale=-0.5)
                if half is None:
                    rs1s[c] = [rs1, 0]
                elif half == 0:
                    rs1s[c] = [rs1, 2]
                else:
                    rs1s[c].append(rs1)

            st = [dict() for _ in range(8)]   # per-pair pipeline state

            def stage_S(p):
                c, pp = p // 2, p % 2
                xc = xcs[c]
                ent = rs1s[c]
                if len(ent) == 2 and ent[1] == 0:
                    rs1, col0 = ent[0], 2 * pp   # whole-chunk rs1 [128,4]
                else:
                    rs1, col0 = ent[0 if pp == 0 else 2], 0  # per-pair [128,2]
                s1 = psS.tile([128, 2, NS], F32, tag=s_tag, name="s1")
                e = []
                for g2 in range(2):
                    g = 2 * pp + g2
                    gsl = slice(g * 128, (g + 1) * 128)
                    for dc in range(DC):
                        nc.tensor.matmul(
                            s1[:, g2, :], xc[:, dc, gsl], k1[:, dc, :],
                            start=(dc == 0), stop=(dc == DC - 1))
                    eg = ep.tile([128, NS], BF16, tag="e", name=f"e1_{g2}")
                    nc.scalar.activation(eg, s1[:, g2, :], Exp,
                                         scale=rs1[:, col0 + g2:
                                                   col0 + g2 + 1])
                    e.append(eg)
                st[p]["e"] = e

            def stage_La(p, li):
                e = st[p]["e"]
                eT = etp.tile([128, 4, 128], BF16, tag="eT")
                if _os2env("TRMODE", "pe") == "dma":
                    # ant-xbar DMA transpose: 16 tiles x 14ns = 224ns on the
                    # DMA device per [128,256] e tile, freeing PE (4 transposes)
                    # and DVE (PSUM->SBUF copy) plus 2 PSUM banks
                    for g2 in range(2):
                        nc.sync.dma_start_transpose(
                            out=eT[:, 2 * g2:2 * g2 + 2, :], in_=e[g2])
                else:
                    tp = psO.tile([128, 4, 128], BF16, tag="o", name="tp")
                    for g2 in range(2):
                        for m in range(NM):
                            nc.tensor.transpose(
                                tp[:, 2 * g2 + m, :],
                                e[g2][:, m * 128:(m + 1) * 128], idm)
                    nc.vector.tensor_copy(eT, tp)
                gps = []
                for g2 in range(2):
                    gp = psG.tile([128, 512], F32, tag="g",
                                  name=f"g_{li}_{g2}")
                    for m in range(NM):
                        nc.tensor.matmul(
                            gp, eT[:, 2 * g2 + m, :], mg[:, li, m, :],
                            start=(m == 0), stop=(m == NM - 1))
                    gps.append(gp)
                st[p]["gps"] = gps

            def stage_Lb(p, li):
                import contextlib, os as _os2
                _hp = int(_os2.environ.get("HP", 0))
                hpctx = tc.high_priority(offset=_hp) if _hp else                     contextlib.nullcontext()
                with hpctx:
                    _stage_Lb_body(p, li)

            def _stage_Lb_body(p, li):
                e = st[p]["e"]
                gps = st[p]["gps"]
                n2l = rsp.tile([128, 2], F32, tag="rs", name=f"n2l_{li}")
                for g2 in range(2):
                    scr = scrp.tile([128, NS], BF16, tag="scr")
                    nc.vector.scalar_tensor_tensor(
                        out=scr, in0=e[g2], scalar=1.0,
                        in1=gps[g2][:, NS:512], op0=MUL, op1=MUL,
                        accum_out=n2l[:, g2:g2 + 1])
                lnl = rsp.tile([128, 2], F32, tag="rs", name=f"lnl_{li}")
                nc.scalar.activation(lnl, n2l, Ln)
                rsl = rsp.tile([128, 2], F32, tag="rs", name=f"rsl_{li}")
                nc.scalar.activation(rsl, lnl, Exp, scale=-0.5)
                if li == 2:
                    s4 = rsp.tile([128, 2], F32, tag="rs", name="s4")
                    st[p]["s4"] = s4
                ne = []
                for g2 in range(2):
                    eg = ep.tile([128, NS], BF16, tag="e",
                                 name=f"e_{li}_{g2}")
                    if li < 2:
                        nc.scalar.activation(eg, gps[g2][:, 0:NS], Exp,
                                             scale=rsl[:, g2:g2 + 1])
                    else:
                        nc.scalar.activation(eg, gps[g2][:, 0:NS], Exp,
                                             scale=rsl[:, g2:g2 + 1],
                                             accum_out=st[p]["s4"][:,
                                                                   g2:g2 + 1])
                    ne.append(eg)
                st[p]["e"] = ne

            def stage_O1(p):
                e, s4 = st[p]["e"], st[p]["s4"]
                rd = rsp.tile([128, 2], F32, tag="rs", name="rd")
                nc.vector.reciprocal(rd, s4)
                st[p]["rd"] = rd
                esT = etp.tile([128, 4, 128], BF16, tag="eT", name="esT")
                if _os2env("TRMODE", "pe") == "dma":
                    for g2 in range(2):
                        nc.sync.dma_start_transpose(
                            out=esT[:, 2 * g2:2 * g2 + 2, :], in_=e[g2])
                else:
                    tp2 = psO.tile([128, 4, 128], BF16, tag="o", name="tp2")
                    for g2 in range(2):
                        for m in range(NM):
                            nc.tensor.transpose(
                                tp2[:, 2 * g2 + m, :],
                                e[g2][:, m * 128:(m + 1) * 128], idm)
                    nc.vector.tensor_copy(esT, tp2)
                st[p]["esT"] = esT

            def stage_O2(p):
                c, pp = p // 2, p % 2
                esT, rd = st[p]["esT"], st[p]["rd"]
                for g2 in range(2):
                    g = 2 * pp + g2
                    tk0 = c * TC + g * 128
                    ob = op.tile([128, D], BF16, tag="o")
                    for h in range(2):
                        po = psG.tile([128, 512], F32, tag="g",
                                      name=f"o_{g2}_{h}")
                        for m in range(NM):
                            nc.tensor.matmul(
                                po, esT[:, 2 * g2 + m, :],
                                v4[:, m, h * 512:(h + 1) * 512],
                                start=(m == 0), stop=(m == NM - 1))
                        _pc = _os2env("POOLCP", "")
                        _h0dve = _os2env("H0DVE", "")
                        if (_pc and int(_pc.split(",")[0]) <= p
                                < int(_pc.split(",")[1])
                                and int(_pc.split(",")[2]) in (h, 2)):
                            # NOTE: disabled by default — GPSIMD has no
                            # PSUM port on TRN2, so any Pool op reading po
                            # (PSUM) fails walrus lowering. Works in the
                            # cost model only (would be ~-1.3us).
                            nc.gpsimd.tensor_mul(
                                ob[:, h * 512:(h + 1) * 512], po,
                                rd[:, g2:g2 + 1].broadcast_to([128, 512]))
                        elif h == 0 and _h0dve and (
                                int(_h0dve.split(",")[0]) <= p
                                < int(_h0dve.split(",")[1])):
                            nc.vector.tensor_scalar_mul(
                                ob[:, h * 512:(h + 1) * 512], po,
                                rd[:, g2:g2 + 1])
                        elif h == 0:
                            nc.scalar.activation(
                                ob[:, h * 512:(h + 1) * 512], po,
                                mybir.ActivationFunctionType.Copy,
                                scale=rd[:, g2:g2 + 1])
                        else:
                            nc.vector.tensor_scalar_mul(
                                ob[:, h * 512:(h + 1) * 512], po,
                                rd[:, g2:g2 + 1])
                    if p >= 6 and _os2env("TAILSPLIT", "0") == "1":
                        # tail-latency: issue each half's DMA as soon as its
                        # copy lands instead of waiting for the whole row
                        nc.sync.dma_start(
                            out=out_d.ap()[tk0:tk0 + 128, 0:512],
                            in_=ob[:, 0:512])
                        nc.sync.dma_start(
                            out=out_d.ap()[tk0:tk0 + 128, 512:1024],
                            in_=ob[:, 512:1024])
                    else:
                        nc.sync.dma_start(
                            out=out_d.ap()[tk0:tk0 + 128, :], in_=ob)

            # ---- modulo-scheduled emission: pair p runs stages at slots
            # p..p+4; adjacent slots interleave independent pairs so every
            # engine queue always has ready work behind a waiting op. ----
            import os as _os
            _SCHED = _os.environ.get("SCHED", "merged")
            _ATOMS = {
                "S": lambda p: stage_S(p),
                "A2": lambda p: stage_La(p, 0), "B2": lambda p: stage_Lb(p, 0),
                "A3": lambda p: stage_La(p, 1), "B3": lambda p: stage_Lb(p, 1),
                "A4": lambda p: stage_La(p, 2), "B4": lambda p: stage_Lb(p, 2),
                "O1": lambda p: stage_O1(p), "O2": lambda p: stage_O2(p),
            }
            _GROUPINGS = {
                # 5 slots/pair, stride 1 (the proven layout)
                "merged": ["S", "A2 B2", "A3 B3", "A4 B4", "O1 O2"],
                # 7 slots/pair: La of next layer shares a slot with prior Lb
                "skew7": ["S", "A2", "B2 A3", "B3 A4", "B4", "O1", "O2"],
                "skew6": ["S", "A2", "B2 A3", "B3 A4", "B4 O1", "O2"],
                "split9": ["S", "A2", "B2", "A3", "B3", "A4", "B4", "O1",
                           "O2"],
                "o1merge": ["S", "A2 B2", "A3 B3", "A4 B4 O1", "O2"],
                "smerge": ["S A2", "B2 A3", "B3 A4", "B4 O1", "O2"],
            }
            _GROUPS = [[ _ATOMS[a] for a in grp.split() ]
                       for grp in _GROUPINGS[_SCHED]]
            NSTG = len(_GROUPS)
            _STARTS = [int(v) for v in
                       _os.environ.get("STARTS", "0,1,2,3,4,5,6,6").split(",")]
            _NORMS = [int(v) for v in
                      _os.environ.get("NORMS", "0,2,4,6").split(",")]
            chunk_load(0)
            chunk_norm(0, half=0)
            chunk_load(1)
            for slot in range(max(_STARTS) + NSTG):
                if slot == 1:
                    # deferred past slot 0 so pair 0's score matmuls are not
                    # queued behind n2 matmuls waiting on the second half of
                    # the chunk-0 load
                    chunk_norm(0, half=1)
                for c in range(1, NCH):
                    if _NORMS[c] == slot:
                        chunk_norm(c)
                _order = [(slot - _STARTS[p], p) for p in range(8)
                          if 0 <= slot - _STARTS[p] < NSTG]
                if _os.environ.get("SLOTORD", "fwd") == "rev":
                    _order.sort(reverse=True)
                else:
                    _order.sort()
                for s, p in _order:
                    for fn in _GROUPS[s]:
                        fn(p)
                if slot % 2 == 0 and (slot // 2 + 2) < NCH:
                    chunk_load(slot // 2 + 2)

    nc.compile()
    return nc


def _prep_inputs(x, keys, values):
    xf = np.ascontiguousarray(x.reshape(B * S, D))
    K = keys.astype(np.float64)
    V = values.astype(np.float64)
    k1t = np.ascontiguousarray(keys[0].T).astype(NPBF16)
    mg = np.empty([3, NS, 512], np.float64)
    for li, l in enumerate([1, 2, 3]):
        mg[li, :, :NS] = V[l - 1] @ K[l].T
        mg[li, :, NS:] = V[l - 1] @ V[l - 1].T
    mg = mg.astype(NPBF16)
    v4 = np.ascontiguousarray(values[3]).astype(NPBF16)
    idm = np.eye(128, dtype=NPBF16)
    in_maps = []
    for i in range(NCORES):
        in_maps.append({
            "xt": np.ascontiguousarray(xf[i * T:(i + 1) * T].T).astype(
                NPBF16),
            "k1t": k1t,
            "mg": mg,
            "v4": v4,
            "idm": idm,
        })
    return in_maps


def kernel(x, keys, values, trace=False):
    x = np.asarray(x, dtype=np.float32)
    keys = np.asarray(keys, dtype=np.float32)
    values = np.asarray(values, dtype=np.float32)
    if "nc" not in _CACHE:
        _CACHE["nc"] = _build()
    nc = _CACHE["nc"]
    in_maps = _prep_inputs(x, keys, values)
    res = run_bass_kernel_spmd(nc, in_maps, core_ids=list(range(NCORES)),
                               trace=trace)
    _CACHE["last_result"] = res
    out = np.concatenate(
        [np.asarray(res.results[i]["outt"], dtype=np.float32)
         for i in range(NCORES)], axis=0)
    return np.ascontiguousarray(out.reshape(B, S, D))

